# revision 1
# baseline (speedup 1.0000x reference)
"""AttentionPool (segment softmax-weighted mean pool) Trainium2 kernel.

Math (reference, fp32):
    h = relu(x @ W1 + b1); l = h @ W2 + b2
    w = exp(l - max(l))                       # global max shift
    mean_w = segment_mean(w, batch, B)        # (B, 1)
    denom = mean_w[batch] * N
    out = segment_mean(w * x / (denom + 1e-8), batch)   # (B, D)

The kernel skips the global max shift: with these magnitudes (l in
[-3, 3]) exp() cannot overflow, and the result is identical up to the
1e-8 epsilon term (relative perturbation ~1e-12), so no cross-core
communication is needed at all once nodes are sharded on segment
boundaries.

Sharding: batch is sorted, so core c owns segments [128c, 128(c+1)) and
the contiguous node range covering them (~62.5k nodes). All per-core
data dependence (which segment each node belongs to) is carried by a
per-tile fp8 one-hot mask tensor, so one SPMD program serves all cores.

Inputs per core (host-prepared):
  x_ext  (T*128, 258) f32r : node-major x | validity col | pad col
  xb     (256, T*128) bf16 : d-major (transposed) x for the MLP matmul
  maskw  (128, 128*T) fp8  : per-tile one-hot node->segment-rel matrix

Per 512-node macro-tile (4 x 128-node tiles, per core), software-
pipelined so every engine's inputs were produced >= 1 macro earlier:
  - 2 DMAs: x_ext macro (128 x 1032 f32r), xb macro (128 x 1024 bf16)
  - PE mm1: Ht(128f x 512n) = W1.T @ Xb   (two K=128 chunks, PSUM acc)
  - ACT: Hr = relu(Ht + b1) -> SBUF bf16  (one op)
  - PE mm2 x4: l(128n x 2) = Hr_sub.T @ [W2|0] -> one LP (128 x 8)
  - ACT: e(128n x 4) = exp(l + b2) -> SBUF (one strided op)
  - DVE: A(128 x 512) = maskw_macro * e_bcast (one tensor_tensor, f32r)
  - PE mm_seg x4: S(128seg x 258) += A_sub.T @ x_ext_tile (PSUM acc;
    x_ext col 256 is the validity column -> per-segment sum of e)
Tail: S -> SBUF, per-segment normalize with host 1/cnt terms, DMA out
(128 x 256) rows = segments [128c, 128(c+1)).
"""
import numpy as np
from contextlib import ExitStack

import ml_dtypes

import concourse.bass as bass
import concourse.bacc as bacc
import concourse.mybir as mybir
import concourse.tile as tile
from concourse.bass_utils import run_bass_kernel_spmd

F32 = mybir.dt.float32
F32R = mybir.dt.float32r
BF16 = mybir.dt.bfloat16
FP8 = mybir.dt.float8e4

NCORES = 8
D = 256
B = 1024
SEG_PER_CORE = B // NCORES  # 128
MAC = 4  # tiles per macro
ABLATE = set()  # perf-debug: names of stages to skip
REPEAT = 1  # perf-debug: run the macro loop R times inside one NEFF
USE_XB = True  # True: host-transposed bf16 MLP input; False: PE transposes


def build_nc(T, num_devices=NCORES):
    """Build the SPMD program for T 128-node tiles per core."""
    nc = bacc.Bacc("TRN2", target_bir_lowering=False, debug=False,
                   enable_asserts=False, num_devices=num_devices)

    x_ext = nc.dram_tensor("x_ext", [128, T * 258], F32R, kind="ExternalInput")
    xb_ext = None
    ident_ext = None
    if USE_XB:
        xb_ext = nc.dram_tensor("xb", [128, T * 1024 // MAC], BF16,
                                kind="ExternalInput")
    else:
        ident_ext = nc.dram_tensor("ident", [128, 128], F32R,
                                   kind="ExternalInput")
    w1_ext = nc.dram_tensor("w1", [256, 128],
                            BF16 if USE_XB else F32R,
                            kind="ExternalInput")
    w2p_ext = nc.dram_tensor("w2p", [128, 2], BF16, kind="ExternalInput")
    b1_ext = nc.dram_tensor("b1c", [128, 1], F32, kind="ExternalInput")
    b2_ext = nc.dram_tensor("b2c", [128, 1], F32, kind="ExternalInput")
    mask_ext = nc.dram_tensor("maskw", [128, 128 * T], FP8, kind="ExternalInput")
    zeros_ext = nc.dram_tensor("zeros258", [128, 258], F32R, kind="ExternalInput")
    inv1_ext = nc.dram_tensor("inv1c", [128, 1], F32, kind="ExternalInput")
    inv2_ext = nc.dram_tensor("inv2c", [128, 1], F32, kind="ExternalInput")
    out_ext = nc.dram_tensor("out", [128, 256], F32, kind="ExternalOutput")

    with tile.TileContext(nc) as tc, ExitStack() as ctx:
        const = ctx.enter_context(tc.tile_pool(name="const", bufs=1))
        xpool = ctx.enter_context(tc.tile_pool(name="xpool", bufs=10))
        xbpool = ctx.enter_context(tc.tile_pool(name="xbpool", bufs=6))
        xs = ctx.enter_context(tc.tile_pool(name="xs", bufs=6))
        xtps = None
        if not USE_XB:
            xtps = ctx.enter_context(
                tc.tile_pool(name="xtps", bufs=3, space="PSUM"))
        hrp = ctx.enter_context(tc.tile_pool(name="hrp", bufs=5))
        ep = ctx.enter_context(tc.tile_pool(name="ep", bufs=5))
        ap_pool = ctx.enter_context(tc.tile_pool(name="ap", bufs=5))
        tailp = ctx.enter_context(tc.tile_pool(name="tailp", bufs=1))
        htps = ctx.enter_context(tc.tile_pool(
            name="htps", bufs=3 if USE_XB else 2, space="PSUM"))
        lps = ctx.enter_context(tc.tile_pool(name="lps", bufs=2, space="PSUM"))
        sps = ctx.enter_context(tc.tile_pool(name="sps", bufs=1, space="PSUM"))

        # --- constants ---
        wdt = BF16 if USE_XB else F32R
        w1a = const.tile([128, 128], wdt)
        nc.sync.dma_start(w1a[:], w1_ext[0:128, :])
        w1b = const.tile([128, 128], wdt)
        nc.sync.dma_start(w1b[:], w1_ext[128:256, :])
        ident = None
        if not USE_XB:
            ident = const.tile([128, 128], F32R)
            nc.sync.dma_start(ident[:], ident_ext[:, :])
        w2p = const.tile([128, 2], BF16)
        nc.sync.dma_start(w2p[:], w2p_ext[:, :])
        b1c = const.tile([128, 1], F32)
        nc.sync.dma_start(b1c[:], b1_ext[:, :])
        b2c = const.tile([128, 1], F32)
        nc.sync.dma_start(b2c[:], b2_ext[:, :])
        zeros258 = const.tile([128, 258], F32R)
        nc.sync.dma_start(zeros258[:], zeros_ext[:, :])
        inv1c = const.tile([128, 1], F32)
        nc.sync.dma_start(inv1c[:], inv1_ext[:, :])
        inv2c = const.tile([128, 1], F32)
        nc.sync.dma_start(inv2c[:], inv2_ext[:, :])
        maskw = const.tile([128, 128 * T], FP8)

        # S accumulator: zero it via a start=True matmul with zero weights
        S_ps = sps.tile([128, 258], F32)
        nc.tensor.matmul(S_ps[:, :], zeros258[:, 0:128], zeros258[:, :],
                         start=True, stop=False, skip_group_check=True)

        assert T % MAC == 0
        nmac = T // MAC
        total = REPEAT * nmac
        # Stage lags (in macro ticks) for software pipelining.
        LAG_MM1, LAG_MM2, LAG_SEG = 1, 2, 3
        state = {}
        for vi in range(total + LAG_SEG + 1):
            i = vi % nmac if vi < total else -1
            if vi < total:
                # stream the mask in chunks of 4 macros, 2 macros ahead
                mi = i + 2
                if vi == 0:
                    for pre in range(3):
                        c0 = 128 * MAC * 4 * pre
                        c1 = min(128 * MAC * 4 * (pre + 1), 128 * T)
                        if c0 < c1:
                            nc.sync.dma_start(maskw[:, c0:c1],
                                              mask_ext[:, c0:c1])
                if vi < nmac and mi % 4 == 0 and (mi // 4) >= 3:
                    c0 = 128 * MAC * mi
                    c1 = min(128 * MAC * (mi + 4), 128 * T)
                    if c0 < c1:
                        nc.sync.dma_start(maskw[:, c0:c1], mask_ext[:, c0:c1])
                st = state.setdefault(vi, {})
                XM = xpool.tile([128, 258 * MAC], F32R, tag="x")
                nc.sync.dma_start(
                    XM[:], x_ext[:, 258 * MAC * i:258 * MAC * (i + 1)])
                Xs = [XM[:, 258 * j:258 * (j + 1)] for j in range(MAC)]
                st["X"] = Xs
                if USE_XB:
                    XB = xbpool.tile([128, 1024], BF16, tag="xb")
                    nc.sync.dma_start(
                        XB[:], xb_ext[:, 1024 * i:1024 * (i + 1)])
                    st["XB"] = XB
                else:
                    XT0 = xtps.tile([128, 512], F32R, tag="xt")
                    XT1 = xtps.tile([128, 512], F32R, tag="xt")
                    for j in range(MAC):
                        nc.tensor.transpose(XT0[:, 128 * j:128 * (j + 1)],
                                            Xs[j][:, 0:128], ident[:])
                    for j in range(MAC):
                        nc.tensor.transpose(XT1[:, 128 * j:128 * (j + 1)],
                                            Xs[j][:, 128:256], ident[:])
                    XS0 = xs.tile([128, 512], F32R, tag="xsl")
                    nc.vector.tensor_copy(XS0[:], XT0[:])
                    XS1 = xs.tile([128, 512], F32R, tag="xsl")
                    nc.scalar.copy(XS1[:], XT1[:])
                    st["XS"] = (XS0, XS1)

            k = vi - LAG_MM1
            if 0 <= k < total:
                st = state[k]
                if USE_XB:
                    rh0, rh1 = st["XB"][:, 0:512], st["XB"][:, 512:1024]
                else:
                    rh0, rh1 = st["XS"][0][:], st["XS"][1][:]
                HT = None
                if not {"mm1", "relu"} <= ABLATE:
                    HT = htps.tile([128, 512], F32)
                if "mm1" not in ABLATE:
                    nc.tensor.matmul(HT[:], w1a[:], rh0,
                                     start=True, stop=False)
                    nc.tensor.matmul(HT[:], w1b[:], rh1,
                                     start=False, stop=True)
                HR = None
                if not {"relu", "mm2"} <= ABLATE:
                    HR = hrp.tile([128, 512], BF16)
                if "relu" not in ABLATE:
                    nc.scalar.activation(HR[:], HT[:],
                                         mybir.ActivationFunctionType.Relu,
                                         bias=b1c[:])
                st["HR"] = HR

            k = vi - LAG_MM2
            if 0 <= k < total:
                st = state[k]
                HR = st["HR"]
                LP = None
                if not {"mm2", "exp"} <= ABLATE:
                    LP = lps.tile([128, 2 * MAC], F32)
                if "mm2" not in ABLATE:
                    for j in range(MAC):
                        nc.tensor.matmul(LP[:, 2 * j:2 * j + 2],
                                         HR[:, 128 * j:128 * (j + 1)], w2p[:],
                                         start=True, stop=True,
                                         skip_group_check=True)
                E4 = None
                if not {"exp", "att"} <= ABLATE:
                    E4 = ep.tile([128, MAC], F32)
                if "exp" not in ABLATE:
                    nc.scalar.activation(E4[:], LP[:, 0:2 * MAC:2],
                                         mybir.ActivationFunctionType.Exp,
                                         bias=b2c[:])
                A2 = None
                km = k % nmac
                m_in = maskw[:, 128 * MAC * km:128 * MAC * (km + 1)].rearrange(
                    "p (t n) -> p t n", t=MAC)
                if not {"att", "mmseg"} <= ABLATE:
                    A2 = ap_pool.tile([128, 128 * MAC], F32R, tag="A")
                if "att" not in ABLATE:
                    e_in = E4[:, :, None].broadcast_to([128, MAC, 128])
                    nc.vector.tensor_tensor(
                        out=A2[:].rearrange("p (t n) -> p t n", t=MAC),
                        in0=m_in, in1=e_in, op=mybir.AluOpType.mult)
                st["A2"] = A2

            k = vi - LAG_SEG
            if 0 <= k < total:
                st = state.pop(k)
                A2 = st["A2"]
                if "mmseg" not in ABLATE:
                    for j in range(MAC):
                        nc.tensor.matmul(S_ps[:, :],
                                         A2[:, 128 * j:128 * (j + 1)],
                                         st["X"][j][:, :],
                                         start=False,
                                         stop=(k == total - 1 and j == MAC - 1),
                                         skip_group_check=True)

        # ---- tail: normalize ----
        S_sb = tailp.tile([128, 258], F32)
        nc.vector.tensor_copy(S_sb[:], S_ps[:])
        dvec = tailp.tile([128, 1], F32)
        nc.vector.tensor_scalar(out=dvec[:], in0=S_sb[:, 256:257],
                                scalar1=inv1c[:], scalar2=1e-8,
                                op0=mybir.AluOpType.mult,
                                op1=mybir.AluOpType.add)
        rvec = tailp.tile([128, 1], F32)
        nc.vector.reciprocal(rvec[:], dvec[:])
        scl = tailp.tile([128, 1], F32)
        nc.vector.tensor_mul(scl[:], rvec[:], inv2c[:])
        out_sb = tailp.tile([128, 256], F32)
        nc.vector.tensor_scalar(out=out_sb[:], in0=S_sb[:, 0:256],
                                scalar1=scl[:], scalar2=None,
                                op0=mybir.AluOpType.mult)
        nc.sync.dma_start(out_ext[:, :], out_sb[:])

    nc.compile()
    return nc


def host_prep(x, batch, W1, b1, W2, b2, ncores=NCORES):
    """Shard on segment boundaries and build per-core input maps."""
    x = np.ascontiguousarray(np.asarray(x, dtype=np.float32))
    batch = np.asarray(batch).astype(np.int64)
    W1 = np.asarray(W1, dtype=np.float32)
    b1 = np.asarray(b1, dtype=np.float32)
    W2 = np.asarray(W2, dtype=np.float32)
    b2 = np.asarray(b2, dtype=np.float32)
    N = x.shape[0]

    sizes = np.bincount(batch, minlength=B)
    starts = np.zeros(B + 1, np.int64)
    starts[1:] = np.cumsum(sizes)
    spc = B // ncores

    T = 0
    for c in range(ncores):
        n = int(starts[spc * (c + 1)] - starts[spc * c])
        T = max(T, (n + 127) // 128)
    T += (-T) % MAC

    w1_np = W1.astype(ml_dtypes.bfloat16) if USE_XB else W1
    w2p_np = np.zeros((128, 2), ml_dtypes.bfloat16)
    w2p_np[:, 0] = W2[:, 0].astype(ml_dtypes.bfloat16)
    b1_np = b1.reshape(128, 1).astype(np.float32)
    b2_np = np.full((128, 1), float(np.asarray(b2).reshape(-1)[0]), np.float32)
    zeros258 = np.zeros((128, 258), np.float32)

    in_maps = []
    for c in range(ncores):
        lo, hi = int(starts[spc * c]), int(starts[spc * (c + 1)])
        n = hi - lo
        xe = np.zeros((T * 128, 258), np.float32)
        xe[:n, 0:256] = x[lo:hi]
        xe[:n, 256] = 1.0
        # swizzle: partition p holds its own rows contiguously
        xe = np.ascontiguousarray(
            xe.reshape(T, 128, 258).transpose(1, 0, 2).reshape(128, T * 258))

        nm = T // MAC
        xbm = None
        if USE_XB:
            xbm = np.zeros((256, T * 128), ml_dtypes.bfloat16)
            xbm[:, :n] = x[lo:hi].T.astype(ml_dtypes.bfloat16)
            # (c p) (i n) -> p (i c n)
            xbm = np.ascontiguousarray(
                xbm.reshape(2, 128, nm, 512).transpose(1, 2, 0, 3).reshape(
                    128, nm * 1024))

        local = (batch[lo:hi] - spc * c).astype(np.int64)
        assert n == 0 or (local.min() >= 0 and local.max() < spc)

        # one-hot mask: maskw[p, 128*t + s] = 1 iff node (t*128+p) in seg s
        maskw = np.zeros((128, 128 * T), ml_dtypes.float8_e4m3)
        pos = np.arange(n)
        tile_of = pos // 128
        maskw[pos % 128, 128 * tile_of + local] = 1.0

        cnt = sizes[spc * c: spc * (c + 1)].astype(np.float32)
        cntc = np.maximum(cnt, 1.0)
        inv1 = (np.float32(N) / cntc).reshape(128, 1).astype(np.float32)
        inv2 = (1.0 / cntc).reshape(128, 1).astype(np.float32)

        im = {
            "x_ext": xe,
            "w1": w1_np,
            "w2p": w2p_np,
            "b1c": b1_np,
            "b2c": b2_np,
            "maskw": maskw,
            "zeros258": zeros258,
            "inv1c": inv1,
            "inv2c": inv2,
        }
        if USE_XB:
            im["xb"] = xbm
        else:
            im["ident"] = np.eye(128, dtype=np.float32)
        im.update()
        in_maps.append(im)
    return T, in_maps


_NC_CACHE = {}
_LAST_RESULTS = None
RUN_KWARGS = {}


def kernel(x, batch, W1, b1, W2, b2):
    global _LAST_RESULTS
    import os
    T, in_maps = host_prep(x, batch, W1, b1, W2, b2)
    key = T
    if key not in _NC_CACHE:
        _NC_CACHE[key] = build_nc(T)
    nc = _NC_CACHE[key]
    kw = dict(RUN_KWARGS)
    if os.environ.get("BASS_KERNEL_TRACE"):
        kw.setdefault("trace", True)
    res = run_bass_kernel_spmd(nc, in_maps, list(range(NCORES)), **kw)
    _LAST_RESULTS = res
    out = np.concatenate([res.results[c]["out"] for c in range(NCORES)], axis=0)
    return out.astype(np.float32)

